# revision 1
# baseline (speedup 1.0000x reference)
"""Akima spline evaluation (nn_Akima_66623532696299) on 8 Trainium2 cores.

Strategy: data-parallel over the batch axis (8 batches per core). The akima
spline y(u), u = 63*x, is a C1 piecewise cubic with integer knots; it is
approximated by a C0 piecewise quadratic in telescoped *windowed-clamp* form

    y(u) = CONST + sum_j  qa_j*d_j + qb_j*d_j^2,   d_j = clamp(u - k_j, 0, w_j)

with one knot per original segment plus extra half/quarter-width knots on
segments with large cubic content (endpoint-pinned Chebyshev reduction keeps
node values exact, so error never accumulates; per-segment error is
|C_k|*0.0481*w^3 <= ~5e-3 absolute, ~4x inside the harness gate).

Engine mapping per knot (measured per-op costs on this HW):
  - DVE tensor_scalar   c = min(max(u, k), k+w)        (~0.3 cyc/elem, 4x mode)
  - ACT Square          s = (sqrt|qb|*(c + phi - k))^2  (completing the square
    folds the linear term into the ACT's free scale+bias; 1 cyc/elem)
  - accumulate sign(qb)*s: split between DVE scalar_tensor_tensor (1 cyc/elem)
    and PE identity-matmul PSUM accumulation (fp32, exact), balancing the
    three engines. Knots with |phi| too large degrade to linear-only knots
    accumulated via stt on c directly.
All per-knot constants are baked as instruction immediates from `value` on
the host in float64 (cached per value). ~3x faster than the telescoped-cubic
baseline which needed 4 DVE + 1 ACT + 4 PE-fp32 passes per segment.
"""

import numpy as np

N_CORES = 8
P = 128
B, CH, H, W = 64, 3, 512, 512
PER_CORE = (B // N_CORES) * CH * H * W        # 6291456
FTOT = PER_CORE // P                          # 49152
TF = 2048                                     # tile free size
NT = FTOT // TF                               # 24 tiles
NSEG = 63
TARGET_ABS = 5.0e-3                           # per-segment error cap
MAXL = 3                                      # max refinement level
PHI_MAX = 4.0                                 # completing-the-square cap

_CACHE = {}
_QBIAS_CACHE = {0: None}
LAST_EXEC_NS = None


def _apply_walrus_compat_patches():
    """This container's walrus rejects >1 sync-wait command per instruction;
    Tile's wait assignment can emit several. Split excess waits onto bare
    same-engine NoOps committed immediately before the instruction."""
    import concourse.tile as tile
    from concourse import mybir
    from concourse.vector_clock import ScopedClock

    if getattr(tile.TileContext, "_akima_patched", False):
        return
    MAX_WAITS = 1
    _orig_commit = tile.TileContext._commit_instruction

    def _split_waits(self, inst, lazy_reg_writes=True):
        si = inst.sync_info
        if si is not None and si.on_wait and len(si.on_wait) > MAX_WAITS:
            waits = list(si.on_wait)
            updates = list(si.on_update or [])
            inst.sync_info = mybir.SyncInfo(on_wait=waits[:MAX_WAITS], on_update=updates)
            for i in range(MAX_WAITS, len(waits), MAX_WAITS):
                nop = mybir.InstNoOp(name=f"I-{self.nc.next_id()}", engine=inst.engine)
                nop.sync_info = mybir.SyncInfo(on_wait=waits[i : i + MAX_WAITS], on_update=[])
                _orig_commit(self, nop, lazy_reg_writes)
        return _orig_commit(self, inst, lazy_reg_writes)

    def _drain_and_barrier(self, tick_clock, wait_clock):
        nc = self.nc
        collector = nc.sync.nop(nofuse=True).ins
        wait_clock.add_sem_waits(collector, ScopedClock({None: tick_clock.global_clock}))
        si = collector.sync_info
        waits = list(si.on_wait or []) if si is not None else []
        updates = list(si.on_update or []) if si is not None else []
        if len(waits) > MAX_WAITS:
            collector.sync_info = mybir.SyncInfo(on_wait=waits[:MAX_WAITS], on_update=updates)
            rest = waits[MAX_WAITS:]
            while rest:
                extra = nc.sync.nop(nofuse=True).ins
                extra.sync_info = mybir.SyncInfo(on_wait=rest[:MAX_WAITS], on_update=[])
                rest = rest[MAX_WAITS:]
        nc.sync.drain()
        nc.all_engine_barrier()
        assert self.sems is not None
        popped = nc._tile_sem_poison_stack.pop()
        assert popped is self._sem_poison
        nc.clear_and_free_semaphores(list(self.sems.allocated().values()))
        nc.all_engine_barrier()

    tile.TileContext._commit_instruction = _split_waits
    tile.TileContext._drain_and_barrier = _drain_and_barrier
    tile.TileContext._akima_patched = True


# ---------------------------------------------------------------------------
# Host-side plan from `value` (float64): windowed-clamp quadratic knots.
# ---------------------------------------------------------------------------
def _akima_locals(value):
    v = np.asarray(value, dtype=np.float64)
    n = v.shape[0]
    h = 1.0 / (n - 1)
    m = np.diff(v) / h
    m_ext = np.concatenate(
        [[3 * m[0] - 2 * m[1], 2 * m[0] - m[1]], m,
         [2 * m[-1] - m[-2], 3 * m[-1] - 2 * m[-2]]]
    )
    dm = np.abs(np.diff(m_ext))
    w1 = dm[2:]
    w2 = dm[:-2]
    den = w1 + w2
    safe = np.where(den > 0, den, 1.0)
    ml = m_ext[1 : n + 1]
    mr = m_ext[2 : n + 2]
    t = np.where(den > 0, (w1 * ml + w2 * mr) / safe, 0.5 * (ml + mr))
    # local cubic per segment (u-units): g_k(d) = A d + B d^2 + C d^3, d in [0,1]
    A = t[:-1] * h
    Bc = (3 * m - 2 * t[:-1] - t[1:]) * h
    Cc = (t[:-1] + t[1:] - 2 * m) * h
    return float(v[0]), A, Bc, Cc


def _plan2(value):
    """Knot list: each knot is a dict with
       kind: 'quad' (scale/bias/sign for ACT-square + acc) or 'lin' (stt on c)
       kappa, w: clamp window [kappa, kappa+w]
       quad: scale, act_bias, sign; lin: coef
    plus the global CONST (v0 + completing-the-square constants)."""
    v0, A, Bc, Cc = _akima_locals(value)
    CHEB = 1.0 / (12.0 * np.sqrt(3.0))  # max |d(d-w)(d-w/2)| / w^3 on [0,w]
    knots = []
    const = v0
    for k in range(len(A)):
        a, b, c = float(A[k]), float(Bc[k]), float(Cc[k])
        if abs(a) + abs(b) + abs(c) < 1e-12:
            continue  # exactly flat segment (relu'd zeros)
        L = 0
        while L < MAXL and CHEB * abs(c) / (8.0 ** L) > TARGET_ABS:
            L += 1
        S = 1 << L
        w = 1.0 / S
        for j in range(S):
            d0 = j / S
            c1 = a + 2 * b * d0 + 3 * c * d0 * d0
            c2 = b + 3 * c * d0
            # endpoint-pinned Chebyshev reduction of the cubic on [0,w]:
            # d^3 ~ d(d-w)(d-w/2) + 1.5w d^2 - 0.5w^2 d
            qa = c1 - 0.5 * c * w * w
            qb = c2 + 1.5 * c * w
            kappa = k + j / S
            if abs(qa) + abs(qb) < 1e-12:
                continue
            phi = qa / (2 * qb) if qb != 0.0 else np.inf
            if np.isfinite(phi) and abs(phi) <= PHI_MAX:
                scale = np.sqrt(abs(qb))
                s_max = abs(qb) * max(phi * phi, (w + phi) * (w + phi))
                knots.append(dict(
                    kind="quad", kappa=kappa, w=w,
                    scale=float(scale), act_bias=float(scale * (phi - kappa)),
                    sign=1.0 if qb > 0 else -1.0, s_max=float(s_max),
                ))
                const -= qb * phi * phi
            else:
                # linear-dominated: absorb qb exactly at both endpoints;
                # the knot accumulates coef*c = coef*(kappa + d), so the
                # coef*kappa constant folds into CONST.
                coef = qa + qb * w
                knots.append(dict(kind="lin", kappa=kappa, w=w, coef=float(coef)))
                const -= coef * kappa
    return float(const), knots


# ---------------------------------------------------------------------------
# Bass kernel builder: 3-engine pipeline.
# ---------------------------------------------------------------------------
def _assign(knots):
    """Assign each knot an accumulation engine:
       'pe_r'  - PE fp32r identity matmul (fast, ~10-bit rounding: only knots
                 whose bounded square values keep total rounding noise small)
       'stt'   - DVE scalar_tensor_tensor (fp32 exact)
       'pe_f'  - (linear knots) PE fp32 with per-knot qa*eye weights, exact
    balancing measured per-tile engine costs."""
    nq = sum(1 for k in knots if k["kind"] == "quad")
    nl = len(knots) - nq
    # fp32r precision budget: sum of (s_max * 2^-11)^2 <= (2e-3)^2
    order = sorted((i for i, k in enumerate(knots) if k["kind"] == "quad"),
                   key=lambda i: knots[i]["s_max"])
    budget = (2.0e-3 * 2048.0) ** 2  # sum s_max^2 cap
    acc2 = 0.0
    pe_r = set()
    for i in order:
        s2 = knots[i]["s_max"] ** 2
        if acc2 + s2 > budget:
            break
        acc2 += s2
        pe_r.add(i)
    if len(pe_r) == nq and nq > 0:
        pe_r.discard(order[-1])  # keep >=1 stt knot to carry CONST
    # linear knots: put on PE(fp32) while projected PE load < ACT load
    act_load = 3.404 * nq
    pe_load = 1.22 * len(pe_r)
    dve_load = 1.255 * len(knots) + 5.292 * 0
    assign = {}
    for i, k in enumerate(knots):
        if k["kind"] == "quad":
            assign[i] = "pe_r" if i in pe_r else "stt"
    for i, k in enumerate(knots):
        if k["kind"] == "lin":
            if pe_load + 8.7 <= act_load:
                assign[i] = "pe_f"
                pe_load += 8.7
            else:
                assign[i] = "stt"
    if not any(v == "stt" for v in assign.values()):
        assign[0] = "stt"
    return assign


def _build_bass(const, knots, reps=1, act_f32r=True, assign=None):
    import concourse.bass as bass
    import concourse.tile as tile
    from concourse import mybir
    from collections import deque

    AL = mybir.AluOpType
    AF = mybir.ActivationFunctionType
    F32 = mybir.dt.float32
    F32R = mybir.dt.float32r
    nc = bass.Bass()
    x = nc.declare_dram_parameter("x", [P, FTOT], F32, isOutput=False)
    eye = nc.declare_dram_parameter("eye", [P, P], F32, isOutput=False)
    qb = nc.declare_dram_parameter("qb", [P, max(1, sum(1 for k in knots if k["kind"] == "quad"))], F32, isOutput=False)
    y = nc.declare_dram_parameter("y", [P, FTOT], F32, isOutput=True)
    NCH = TF // 512

    if assign is None:
        assign = _assign(knots)
    pe_knots = [i for i in range(len(knots)) if assign[i] in ("pe_r", "pe_f")]
    pe_total = len(pe_knots)
    lin_pe = [i for i in pe_knots if assign[i] == "pe_f"]
    # act-bias column index per quad knot
    qcol = {}
    for i, kn in enumerate(knots):
        if kn["kind"] == "quad":
            qcol[i] = len(qcol)

    with tile.TileContext(nc) as tc:
        with (
            tc.tile_pool(name="cw", bufs=1) as cw,
            tc.tile_pool(name="xp", bufs=2) as xp,
            tc.tile_pool(name="cp2", bufs=3) as cpool,
            tc.tile_pool(name="spr", bufs=2) as spool_r,
            tc.tile_pool(name="spf", bufs=3) as spool_f,
            tc.tile_pool(name="yp", bufs=2) as ypool,
            tc.tile_pool(name="ap", bufs=2) as apool,
            tc.tile_pool(name="ps", bufs=1, space="PSUM") as psp,
        ):
            eyep = cw.tile([P, P], F32, tag="eyep")
            nc.sync.dma_start(eyep[:], eye[:])
            qbt = cw.tile([P, max(1, len(qcol))], F32, tag="qbt")
            nc.sync.dma_start(qbt[:], qb[:])
            eyer_p = cw.tile([P, P], F32R, tag="eyer_p")
            nc.vector.tensor_scalar(out=eyer_p[:], in0=eyep[:], scalar1=1.0, scalar2=None, op0=AL.mult)
            eyer_n = cw.tile([P, P], F32R, tag="eyer_n")
            nc.vector.tensor_scalar(out=eyer_n[:], in0=eyep[:], scalar1=-1.0, scalar2=None, op0=AL.mult)
            lin_w = {}
            for i in lin_pe:
                wt = cw.tile([P, P], F32, tag=f"w{i}")
                nc.vector.tensor_scalar(out=wt[:], in0=eyep[:],
                                        scalar1=float(knots[i]["coef"]), scalar2=None, op0=AL.mult)
                lin_w[i] = wt

            for it in [i % NT for i in range(NT * reps)]:
                xt = xp.tile([P, TF], F32, tag="xt")
                nc.sync.dma_start(xt[:], x[:, bass.ts(it, TF)])
                # u = 63*x in place
                nc.vector.tensor_scalar(out=xt[:], in0=xt[:], scalar1=63.0, scalar2=None, op0=AL.mult)

                acc = apool.tile([P, TF], F32, tag="acc")
                if pe_total:
                    pacc = psp.tile([P, TF], F32, tag="pacc")
                else:
                    pacc = None
                pe_seen = 0
                pending = deque()  # (src_tile, coef): DVE accs lag a few knots
                first_dve_acc = True

                def flush_one():
                    nonlocal first_dve_acc
                    st, coef = pending.popleft()
                    if first_dve_acc:
                        nc.vector.tensor_scalar(
                            out=acc[:], in0=st[:], scalar1=coef, scalar2=float(const),
                            op0=AL.mult, op1=AL.add)
                        first_dve_acc = False
                    else:
                        nc.vector.scalar_tensor_tensor(
                            out=acc[:], in0=st[:], scalar=coef, in1=acc[:],
                            op0=AL.mult, op1=AL.add)

                for i, kn in enumerate(knots):
                    kap, w = kn["kappa"], kn["w"]
                    mode = assign[i]
                    ct = cpool.tile([P, TF], F32, tag="ct")
                    nc.vector.tensor_scalar(
                        out=ct[:], in0=xt[:], scalar1=float(kap), scalar2=float(kap + w),
                        op0=AL.max, op1=AL.min)
                    if kn["kind"] == "quad":
                        if mode == "pe_r":
                            st = spool_r.tile([P, TF], F32R, tag="str")
                            bias_ap = qbt[:, qcol[i] : qcol[i] + 1]
                            if act_f32r:
                                nc.scalar.activation(st[:], ct[:], AF.Square,
                                                     bias=bias_ap,
                                                     scale=float(kn["scale"]))
                            else:
                                sf = spool_f.tile([P, TF], F32, tag="stf")
                                nc.scalar.activation(sf[:], ct[:], AF.Square,
                                                     bias=bias_ap,
                                                     scale=float(kn["scale"]))
                                nc.vector.tensor_scalar(out=st[:], in0=sf[:],
                                                        scalar1=1.0, scalar2=None, op0=AL.mult)
                            pe_seen += 1
                            eyet = eyer_p if kn["sign"] > 0 else eyer_n
                            for jch in range(NCH):
                                nc.tensor.matmul(
                                    pacc[:, bass.ts(jch, 512)], eyet[:],
                                    st[:, bass.ts(jch, 512)],
                                    start=(pe_seen == 1), stop=(pe_seen == pe_total),
                                )
                        else:
                            st = spool_f.tile([P, TF], F32, tag="st")
                            nc.scalar.activation(st[:], ct[:], AF.Square,
                                                 bias=qbt[:, qcol[i] : qcol[i] + 1],
                                                 scale=float(kn["scale"]))
                            pending.append((st, float(kn["sign"])))
                    else:
                        if mode == "pe_f":
                            pe_seen += 1
                            for jch in range(NCH):
                                nc.tensor.matmul(
                                    pacc[:, bass.ts(jch, 512)], lin_w[i][:],
                                    ct[:, bass.ts(jch, 512)],
                                    start=(pe_seen == 1), stop=(pe_seen == pe_total),
                                )
                        else:
                            pending.append((ct, float(kn["coef"])))
                    while len(pending) > 2:
                        flush_one()
                while pending:
                    flush_one()

                if pe_total:
                    yt = ypool.tile([P, TF], F32, tag="yt")
                    nc.scalar.activation(yt[:], pacc[:], AF.Copy, bias=0.0, scale=1.0)
                    nc.vector.tensor_tensor(out=acc[:], in0=acc[:], in1=yt[:], op=AL.add)
                nc.sync.dma_start(y[:, bass.ts(it, TF)], acc[:])
    return nc


# ---------------------------------------------------------------------------
# Baseline fallback (the original telescoped-cubic kernel).
# ---------------------------------------------------------------------------
def _build_bass_baseline_nc(A, Bc, Cc, v0, reps=1):
    import concourse.bass as bass
    import concourse.tile as tile
    from concourse import mybir

    AL = mybir.AluOpType
    AF = mybir.ActivationFunctionType
    F32 = mybir.dt.float32
    nc = bass.Bass()
    x = nc.declare_dram_parameter("x", [P, FTOT], F32, isOutput=False)
    eye = nc.declare_dram_parameter("eye", [P, P], F32, isOutput=False)
    kb = nc.declare_dram_parameter("kb", [P, NSEG], F32, isOutput=False)
    y = nc.declare_dram_parameter("y", [P, FTOT], F32, isOutput=True)
    TFB = 2048
    NTB = FTOT // TFB
    NCHUNK = TFB // 512

    with tile.TileContext(nc) as tc:
        with (
            tc.tile_pool(name="cp", bufs=1) as cp,
            tc.tile_pool(name="xp", bufs=2) as xp,
            tc.tile_pool(name="rp", bufs=3) as rp,
            tc.tile_pool(name="pp", bufs=3) as pp,
            tc.tile_pool(name="gp", bufs=4) as gp,
            tc.tile_pool(name="op", bufs=2) as op,
            tc.tile_pool(name="ps", bufs=2, space="PSUM") as ps,
        ):
            eyet = cp.tile([P, P], F32, tag="eye")
            nc.sync.dma_start(eyet[:], eye[:])
            kbt = cp.tile([P, NSEG], F32, tag="kbt")
            nc.sync.dma_start(kbt[:], kb[:])
            for it in [i % NTB for i in range(NTB * reps)]:
                xt = xp.tile([P, TFB], F32, tag="xt")
                nc.sync.dma_start(xt[:], x[:, bass.ts(it, TFB)])
                acc = ps.tile([P, TFB], F32, tag="acc")
                for k in range(NSEG):
                    a, b, c = float(A[k]), float(Bc[k]), float(Cc[k])
                    r = rp.tile([P, TFB], F32, tag="r")
                    # ACT: r = relu(63*x - k); DVE: d = min(r, 1)
                    nc.scalar.activation(r[:], xt[:], AF.Relu, bias=kbt[:, k : k + 1], scale=63.0)
                    nc.vector.tensor_scalar(out=r[:], in0=r[:], scalar1=1.0, scalar2=None, op0=AL.min)
                    p = pp.tile([P, TFB], F32, tag="p")
                    # p = d*C + B ; q = p*d ; g = (q + A)*d
                    nc.vector.tensor_scalar(out=p[:], in0=r[:], scalar1=c, scalar2=b, op0=AL.mult, op1=AL.add)
                    nc.vector.tensor_tensor(out=p[:], in0=p[:], in1=r[:], op=AL.mult)
                    g = gp.tile([P, TFB], F32, tag="g")
                    nc.vector.scalar_tensor_tensor(out=g[:], in0=p[:], scalar=a, in1=r[:], op0=AL.add, op1=AL.mult)
                    # PE: acc (PSUM) += eye.T @ g, one matmul per 512-wide bank
                    for j in range(NCHUNK):
                        nc.tensor.matmul(
                            acc[:, bass.ts(j, 512)], eyet[:], g[:, bass.ts(j, 512)],
                            start=(k == 0), stop=(k == NSEG - 1),
                        )
                yt = op.tile([P, TFB], F32, tag="yt")
                nc.scalar.activation(yt[:], acc[:], AF.Copy, bias=float(v0), scale=1.0)
                nc.sync.dma_start(y[:, bass.ts(it, TFB)], yt[:])
    return nc


def _build_bass_baseline(value, reps=1):
    v0_, A64, B64, C64 = _akima_locals(value)
    return _build_bass_baseline_nc(
        np.asarray(A64, np.float32), np.asarray(B64, np.float32),
        np.asarray(C64, np.float32), np.float32(v0_), reps=reps)


def kernel(input, value):
    global LAST_EXEC_NS
    import time

    _apply_walrus_compat_patches()
    from concourse.bass_utils import run_bass_kernel_spmd

    input = np.ascontiguousarray(np.asarray(input, dtype=np.float32))
    value = np.ascontiguousarray(np.asarray(value, dtype=np.float32))
    key = value.tobytes()
    shards = input.reshape(N_CORES, P, FTOT)
    eyev = np.eye(P, dtype=np.float32)

    def _in_maps(nc):
        names = set()
        from concourse import mybir as _mb
        for alloc in nc.m.functions[0].allocations:
            if isinstance(alloc, _mb.MemoryLocationSet) and alloc.kind == "ExternalInput":
                names.add(alloc.memorylocations[0].name)
        maps = []
        for c in range(N_CORES):
            m = {"x": shards[c]}
            if "eye" in names:
                m["eye"] = eyev
            if "qb" in names:
                m["qb"] = _QBIAS_CACHE[0]
            if "kb" in names:
                m["kb"] = np.broadcast_to(-np.arange(NSEG, dtype=np.float32), (P, NSEG)).copy()
            maps.append(m)
        return maps

    def _run(nc):
        global LAST_EXEC_NS
        t0 = time.time()
        res = run_bass_kernel_spmd(nc, _in_maps(nc), core_ids=list(range(N_CORES)))
        LAST_EXEC_NS = (time.time() - t0) * 1e9
        out = np.stack([res.results[c]["y"] for c in range(N_CORES)], axis=0)
        return out.reshape(B, CH, H, W).astype(np.float32, copy=False)

    entry = _CACHE.get(key)
    if entry is not None:
        return _run(entry)

    candidates = []
    try:
        const, knots = _plan2(value)
        qb_vals = np.asarray([k["act_bias"] for k in knots if k["kind"] == "quad"],
                             dtype=np.float32)
        if qb_vals.size == 0:
            qb_vals = np.zeros(1, np.float32)
        _QBIAS_CACHE[0] = np.broadcast_to(qb_vals, (P, qb_vals.size)).copy()
        candidates.append(lambda: _build_bass(const, knots, act_f32r=True))
        candidates.append(lambda: _build_bass(const, knots, act_f32r=False))
    except Exception:
        pass
    candidates.append(lambda: _build_bass_baseline(value))
    last_exc = None
    for mk in candidates:
        try:
            nc = mk()
            out = _run(nc)
            _CACHE.clear()
            _CACHE[key] = nc
            return out
        except Exception as e:  # compile/run failure: fall through
            last_exc = e
    raise last_exc



# revision 2
# speedup vs baseline: 2.0007x; 2.0007x over previous
"""Akima spline evaluation (nn_Akima_66623532696299) on 8 Trainium2 cores.

v2: fp16 windowed-knot decomposition, engine-balanced:

    y(x) = const + sum_j f_j(x16),   x16 = fp16(x) (gpsimd cast-DMA, RNE)
      lin knot:  f = g16 * clamp(x16, a, b)          DVE clamp (4x mode) +
                                                     PE fp16 diag-matmul acc
      quad knot: f = sgn * fp16((sig*c + bias)^2)    + ACT Square
    PSUM accumulates all knots in fp32; ACT drains PSUM + adds const; output
    written back as fp32 by a gpsimd casting DMA.

The host planner covers the 63 spline segments with pieces (1, 2, 0.5 or 0.25
segments wide, fp16 breakpoints) chosen by DP to minimize a per-engine cost,
with a per-piece error budget = TAU minus the local fp16 input-rounding
slack. The complete device pipeline (fp16 rounding included) is emulated
exactly on the full fp16 grid on the host; if the bound exceeds the safety
threshold the kernel falls back to the slower exact-telescoped v1 kernels.
"""

import numpy as np

N_CORES = 8
P = 128
B, CH, H, W = 64, 3, 512, 512
PER_CORE = (B // N_CORES) * CH * H * W        # 6291456
FTOT = PER_CORE // P                          # 49152
TF = 2048
NT = FTOT // TF                               # 24
NSEG = 63
TAU = 0.015
S_CAP = 4.0
ERR_GATE = 0.019

# per-op per-tile cost model ns (TF=2048) for engine balancing
C_CLAMP_DVE = 654.0
C_ACC_PE = 864.0
C_ACC_DVE = 2254.0
C_SQ_ACT = 2077.0
C_MERGE_DVE = 1187.0
C_CONV_DVE = 1187.0

_CACHE = {}
_QBIAS_CACHE = {0: None}
LAST_EXEC_NS = None


def _apply_walrus_compat_patches():
    """This container's walrus rejects >1 sync-wait command per instruction;
    Tile's wait assignment can emit several. Split excess waits onto bare
    same-engine NoOps committed immediately before the instruction."""
    import concourse.tile as tile
    from concourse import mybir
    from concourse.vector_clock import ScopedClock

    if getattr(tile.TileContext, "_akima_patched", False):
        return
    MAX_WAITS = 1
    _orig_commit = tile.TileContext._commit_instruction

    def _split_waits(self, inst, lazy_reg_writes=True):
        si = inst.sync_info
        if si is not None and si.on_wait and len(si.on_wait) > MAX_WAITS:
            waits = list(si.on_wait)
            updates = list(si.on_update or [])
            inst.sync_info = mybir.SyncInfo(on_wait=waits[:MAX_WAITS], on_update=updates)
            for i in range(MAX_WAITS, len(waits), MAX_WAITS):
                nop = mybir.InstNoOp(name=f"I-{self.nc.next_id()}", engine=inst.engine)
                nop.sync_info = mybir.SyncInfo(on_wait=waits[i : i + MAX_WAITS], on_update=[])
                _orig_commit(self, nop, lazy_reg_writes)
        return _orig_commit(self, inst, lazy_reg_writes)

    def _drain_and_barrier(self, tick_clock, wait_clock):
        nc = self.nc
        collector = nc.sync.nop(nofuse=True).ins
        wait_clock.add_sem_waits(collector, ScopedClock({None: tick_clock.global_clock}))
        si = collector.sync_info
        waits = list(si.on_wait or []) if si is not None else []
        updates = list(si.on_update or []) if si is not None else []
        if len(waits) > MAX_WAITS:
            collector.sync_info = mybir.SyncInfo(on_wait=waits[:MAX_WAITS], on_update=updates)
            rest = waits[MAX_WAITS:]
            while rest:
                extra = nc.sync.nop(nofuse=True).ins
                extra.sync_info = mybir.SyncInfo(on_wait=rest[:MAX_WAITS], on_update=[])
                rest = rest[MAX_WAITS:]
        nc.sync.drain()
        nc.all_engine_barrier()
        assert self.sems is not None
        popped = nc._tile_sem_poison_stack.pop()
        assert popped is self._sem_poison
        nc.clear_and_free_semaphores(list(self.sems.allocated().values()))
        nc.all_engine_barrier()

    tile.TileContext._commit_instruction = _split_waits
    tile.TileContext._drain_and_barrier = _drain_and_barrier
    tile.TileContext._akima_patched = True


# ===========================================================================
# v2: host planner
# ===========================================================================
def akima_locals(value):
    v = np.asarray(value, dtype=np.float64)
    n = v.shape[0]
    h = 1.0 / (n - 1)
    m = np.diff(v) / h
    m_ext = np.concatenate(
        [[3 * m[0] - 2 * m[1], 2 * m[0] - m[1]], m,
         [2 * m[-1] - m[-2], 3 * m[-1] - 2 * m[-2]]])
    dm = np.abs(np.diff(m_ext))
    w1 = dm[2:]
    w2 = dm[:-2]
    den = w1 + w2
    safe = np.where(den > 0, den, 1.0)
    ml = m_ext[1:n + 1]
    mr = m_ext[2:n + 2]
    t = np.where(den > 0, (w1 * ml + w2 * mr) / safe, 0.5 * (ml + mr))
    A = t[:-1] * h
    Bc = (3 * m - 2 * t[:-1] - t[1:]) * h
    Cc = (t[:-1] + t[1:] - 2 * m) * h
    return float(v[0]), A, Bc, Cc


def _mk_y(value):
    v0, A, Bc, Cc = akima_locals(value)
    vnodes = np.concatenate([[v0], v0 + np.cumsum(A + Bc + Cc)])

    def y(u):
        u = np.asarray(u, dtype=np.float64)
        uc = np.clip(u, 0.0, 63.0)
        idx = np.clip(np.floor(uc).astype(np.int64), 0, 62)
        d = uc - idx
        return vnodes[idx] + d * (A[idx] + d * (Bc[idx] + d * Cc[idx]))
    return y


def _f16_grid(a, b):
    ia = np.float16(a).view(np.uint16)
    ib = np.float16(b).view(np.uint16)
    codes = np.arange(int(ia), int(ib) + 1, dtype=np.uint16)
    return codes.view(np.float16).astype(np.float64)


def _piece_lin(y, a, b):
    ya, yb = y(63 * a), y(63 * b)
    g = (yb - ya) / (b - a)
    g16 = float(np.float64(np.float16(g)))
    gg = _f16_grid(a, b)
    approx = ya + g16 * (gg - a)
    err = np.abs(approx - y(63 * gg)).max()
    return g16, float(err)


def _piece_quad(y, a, b):
    Wd = b - a
    ya, yb = y(63 * a), y(63 * b)
    dy = yb - ya
    gg = _f16_grid(a, b)
    t = gg - a
    target = y(63 * gg) - ya
    den = (t * t - Wd * t)
    num = target - (dy / Wd) * t
    denom = float((den * den).sum())
    beta0 = float((num * den).sum() / denom) if denom > 1e-30 else 0.0
    best = None
    for beta in np.linspace(beta0 - abs(beta0) - 1.0, beta0 + abs(beta0) + 1.0, 81):
        alpha = (dy - beta * Wd * Wd) / Wd
        if abs(beta) < 1e-9:
            continue
        sig = np.sqrt(abs(beta))
        hshift = alpha / (2 * beta)
        s_max = abs(beta) * max(hshift * hshift, (Wd + hshift) * (Wd + hshift))
        if s_max > S_CAP:
            continue
        sig32 = float(np.float32(sig))
        bias32 = float(np.float32(sig * (hshift - a)))
        sgn = 1.0 if beta > 0 else -1.0
        s = (np.float32(sig32) * gg.astype(np.float32) + np.float32(bias32)).astype(np.float32)
        s16 = (s * s).astype(np.float16).astype(np.float64)
        approx = sgn * s16 - sgn * float(np.float16((sig32 * a + bias32) ** 2))
        err = np.abs(approx - target).max()
        if best is None or err < best[4]:
            best = (sig32, bias32, sgn, float(s_max), float(err))
    return best


def plan(value, tau=TAU, quad_cost=3.2):
    y = _mk_y(value)

    def slack(a, b):
        gg = _f16_grid(a, b)
        if len(gg) < 3:
            return 0.0
        yv = y(63 * gg)
        dy = np.abs(np.diff(yv))
        halfd = np.concatenate([[dy[0]], np.maximum(dy[:-1], dy[1:]), [dy[-1]]]) / 2
        return float(halfd.max())

    def best_single(lo, hi):
        a, b = [float(np.float64(np.float16(lo / 63.0))), float(np.float64(np.float16(hi / 63.0)))]
        budget = tau - slack(a, b)
        if budget <= 0.001:
            return None
        g16, errl = _piece_lin(y, a, b)
        if errl <= budget:
            return 2.2, [dict(kind="lin", a=a, b=b, g=g16)]
        q = _piece_quad(y, a, b)
        if q is not None and q[4] <= budget:
            sig, bias, sgn, smax, _ = q
            return quad_cost, [dict(kind="quad", a=a, b=b, sig=sig, bias=bias, sgn=sgn, smax=smax)]
        return None

    def cover(lo, hi, depth=0):
        s = best_single(lo, hi)
        if s is not None:
            return s
        if depth >= 3:
            a, b = [float(np.float64(np.float16(lo / 63.0))), float(np.float64(np.float16(hi / 63.0)))]
            q = _piece_quad(y, a, b)
            if q is None:
                g16, _ = _piece_lin(y, a, b)
                return 2.2, [dict(kind="lin", a=a, b=b, g=g16)]
            sig, bias, sgn, smax, _ = q
            return 3.2, [dict(kind="quad", a=a, b=b, sig=sig, bias=bias, sgn=sgn, smax=smax)]
        mid = (lo + hi) / 2
        c1, k1 = cover(lo, mid, depth + 1)
        c2, k2 = cover(mid, hi, depth + 1)
        return c1 + c2, k1 + k2

    INF = 1e18
    dp = [(INF, None)] * (NSEG + 1)
    dp[NSEG] = (0.0, None)
    choice = [None] * NSEG
    for k in range(NSEG - 1, -1, -1):
        cs, ks = cover(k, k + 1)
        best = (cs + dp[k + 1][0], ("single", ks))
        if k + 2 <= NSEG:
            p = best_single(k, k + 2)
            if p is not None:
                cp = p[0] + dp[k + 2][0]
                if cp < best[0]:
                    best = (cp, ("pair", p[1]))
        dp[k] = (best[0], best[1])
        choice[k] = best[1]
    knots = []
    k = 0
    while k < NSEG:
        kind, ks = choice[k]
        knots.extend(ks)
        k += 2 if kind == "pair" else 1

    const = y(0.0)
    for kn in knots:
        if kn["kind"] == "lin":
            const -= kn["g"] * kn["a"]
        else:
            const -= kn["sgn"] * float(np.float16((kn["sig"] * kn["a"] + kn["bias"]) ** 2))
    return float(const), knots


def emulate(const, knots, acceng=None):
    gg = _f16_grid(0.0, 1.0)
    n = len(knots)
    if acceng is None:
        acceng = ["pe"] * n
    psum = np.zeros(gg.shape, dtype=np.float64)
    accD = None
    for j, kn in enumerate(knots):
        c = np.clip(gg, kn["a"], kn["b"])
        if kn["kind"] == "lin":
            src = c
            wv_pe = float(np.float64(np.float16(kn["g"])))
            wv_dve = float(np.float32(kn["g"]))
        else:
            s = (np.float32(kn["sig"]) * c.astype(np.float32)
                 + np.float32(kn["bias"])).astype(np.float32)
            src = (s * s).astype(np.float16).astype(np.float64)
            wv_pe = kn["sgn"]
            wv_dve = kn["sgn"]
        if acceng[j] == "pe":
            psum += wv_pe * src
        else:
            if accD is None:
                accD = (src * wv_dve).astype(np.float16).astype(np.float64)
            else:
                accD = (src * wv_dve + accD).astype(np.float16).astype(np.float64)
    yt = (psum + float(np.float32(const))).astype(np.float16).astype(np.float64)
    if accD is not None:
        yt = (yt + accD).astype(np.float16).astype(np.float64)
    return gg, yt


def verify_plan(value, const, knots, acceng=None):
    y = _mk_y(value)
    gg, yh = emulate(const, knots, acceng)
    ytrue = y(63 * gg)
    err_grid = np.abs(yh - ytrue)
    gaps = np.diff(gg)
    half = np.concatenate([[0], gaps]) / 2 + np.concatenate([gaps, [0]]) / 2
    dy = np.abs(np.gradient(ytrue, gg, edge_order=1))
    slack = dy * half
    return float(err_grid.max()), float((err_grid + slack).max())


def assign_engines(knots, conv_on_dve=False):
    n = len(knots)
    nsq = sum(1 for k in knots if k["kind"] == "quad")
    act = (nsq + 1) * C_SQ_ACT
    nlin = n - nsq
    best = None
    for aD in range(0, min(13, nlin + 1)):
        aP = n - aD
        dve = n * C_CLAMP_DVE + aD * C_ACC_DVE + (C_MERGE_DVE if aD else 0.0) \
            + (C_CONV_DVE if conv_on_dve else 0.0)
        pe = aP * C_ACC_PE
        t = max(dve, pe, act)
        if best is None or t < best[0]:
            best = (t, aD)
    t, aD = best
    clampeng = ["dve"] * n
    acceng = ["pe"] * n
    lin_idx = [j for j, k in enumerate(knots) if k["kind"] == "lin"]
    for j in lin_idx[:aD]:
        acceng[j] = "dve"
    return clampeng, acceng, dict(tile_ns=t, dve_accs=aD,
                                  act_ns=act, pe_ns=(n - aD) * C_ACC_PE)


def plan_best(value, tau=TAU):
    best = None
    for qc in (2.6, 3.2, 4.2, 5.6, 7.5):
        const, knots = plan(value, tau=tau, quad_cost=qc)
        ce, ae, stats = assign_engines(knots)
        if best is None or stats["tile_ns"] < best[3]["tile_ns"]:
            best = (const, knots, (ce, ae), stats, qc)
    const, knots, (ce, ae), stats, qc = best
    return const, knots, ce, ae, dict(stats, quad_cost=qc)


def make_tables(knots, acceng):
    pe_knots = [j for j in range(len(knots)) if acceng[j] == "pe"]
    n_pe = len(pe_knots)
    wstack = np.zeros((P, max(1, n_pe) * P), np.float16)
    for i, j in enumerate(pe_knots):
        kn = knots[j]
        wv = kn["sgn"] if kn["kind"] == "quad" else kn["g"]
        blk = wstack[:, i * P:(i + 1) * P]
        np.fill_diagonal(blk, np.float16(wv))
    quads = [j for j in range(len(knots)) if knots[j]["kind"] == "quad"]
    qb = np.zeros((P, max(1, len(quads))), np.float32)
    for i, j in enumerate(quads):
        qb[:, i] = np.float32(knots[j]["bias"])
    return wstack, qb


# ===========================================================================
# v2: bass builder
# ===========================================================================
def build_v2(const, knots, clampeng, acceng, reps=1):
    import concourse.bass as bass
    import concourse.tile as tile
    from concourse import mybir

    AL = mybir.AluOpType
    AF = mybir.ActivationFunctionType
    F32 = mybir.dt.float32
    F16 = mybir.dt.float16

    n = len(knots)
    pe_knots = [j for j in range(n) if acceng[j] == "pe"]
    n_pe = len(pe_knots)
    quads = [j for j in range(n) if knots[j]["kind"] == "quad"]
    qcol = {j: i for i, j in enumerate(quads)}
    pecol = {j: i for i, j in enumerate(pe_knots)}

    nc = bass.Bass()
    x = nc.declare_dram_parameter("x", [P, FTOT], F32, isOutput=False)
    wstack = nc.declare_dram_parameter("wstack", [P, max(1, n_pe) * P], F16, isOutput=False)
    qb = nc.declare_dram_parameter("qb", [P, max(1, len(quads))], F32, isOutput=False)
    y = nc.declare_dram_parameter("y", [P, FTOT], F32, isOutput=True)

    NCH = TF // 512

    with tile.TileContext(nc) as tc:
        with (
            tc.tile_pool(name="cw", bufs=1) as cw,
            tc.tile_pool(name="xp", bufs=3) as xp,
            tc.tile_pool(name="cp", bufs=6) as cpool,
            tc.tile_pool(name="ca", bufs=3) as capool,
            tc.tile_pool(name="sp", bufs=4) as spool,
            tc.tile_pool(name="ap", bufs=2) as apool,
            tc.tile_pool(name="yp", bufs=3) as ypool,
            tc.tile_pool(name="ps", bufs=2, space="PSUM") as psp,
        ):
            wst = cw.tile([P, max(1, n_pe) * P], F16, tag="wst")
            nc.sync.dma_start(wst[:], wstack[:])
            qbt = cw.tile([P, max(1, len(quads))], F32, tag="qbt")
            nc.sync.dma_start(qbt[:], qb[:])

            for it in [i % NT for i in range(NT * reps)]:
                x16 = xp.tile([P, TF], F16, tag="x16")
                nc.gpsimd.dma_start(x16[:], x[:, bass.ts(it, TF)])
                pacc = psp.tile([P, TF], F32, tag="pacc")
                accD = None
                pe_seen = 0
                for j, kn in enumerate(knots):
                    is_dve_acc = acceng[j] == "dve"
                    pool = capool if is_dve_acc else cpool
                    c = pool.tile([P, TF], F16, tag="ca" if is_dve_acc else "c")
                    eng = nc.vector if clampeng[j] == "dve" else nc.gpsimd
                    eng.tensor_scalar(out=c[:], in0=x16[:],
                                      scalar1=float(kn["a"]), scalar2=float(kn["b"]),
                                      op0=AL.max, op1=AL.min)
                    if kn["kind"] == "quad":
                        s = spool.tile([P, TF], F16, tag="s")
                        nc.scalar.activation(s[:], c[:], AF.Square,
                                             bias=qbt[:, qcol[j]:qcol[j] + 1],
                                             scale=float(kn["sig"]))
                        src = s
                        wv = kn["sgn"]
                    else:
                        src = c
                        wv = kn["g"]
                    if acceng[j] == "pe":
                        pj = pecol[j]
                        for jch in range(NCH):
                            nc.tensor.matmul(
                                pacc[:, bass.ts(jch, 512)],
                                wst[:, bass.ts(pj, P)],
                                src[:, bass.ts(jch, 512)],
                                start=(pe_seen == 0), stop=(pe_seen == n_pe - 1))
                        pe_seen += 1
                    else:
                        if accD is None:
                            accD = apool.tile([P, TF], F16, tag="accD")
                            nc.vector.tensor_scalar(out=accD[:], in0=src[:],
                                                    scalar1=float(wv), scalar2=None,
                                                    op0=AL.mult)
                        else:
                            nc.vector.scalar_tensor_tensor(
                                out=accD[:], in0=src[:], scalar=float(wv),
                                in1=accD[:], op0=AL.mult, op1=AL.add)
                yt = ypool.tile([P, TF], F16, tag="yt")
                nc.scalar.activation(yt[:], pacc[:], AF.Copy, bias=float(const),
                                     scale=1.0)
                if accD is not None:
                    nc.vector.tensor_tensor(out=yt[:], in0=yt[:], in1=accD[:],
                                            op=AL.add)
                    accD = None
                nc.gpsimd.dma_start(y[:, bass.ts(it, TF)], yt[:])
    return nc


# ===========================================================================
# v1 fallback kernels (exact-telescoped quadratic / cubic)
# ===========================================================================
def _plan2(value):
    MAXL = 3
    TARGET_ABS = 5.0e-3
    PHI_MAX = 4.0
    v0, A, Bc, Cc = akima_locals(value)
    CHEB = 1.0 / (12.0 * np.sqrt(3.0))
    knots = []
    const = v0
    for k in range(len(A)):
        a, b, c = float(A[k]), float(Bc[k]), float(Cc[k])
        if abs(a) + abs(b) + abs(c) < 1e-12:
            continue
        L = 0
        while L < MAXL and CHEB * abs(c) / (8.0 ** L) > TARGET_ABS:
            L += 1
        S = 1 << L
        w = 1.0 / S
        for j in range(S):
            d0 = j / S
            c1 = a + 2 * b * d0 + 3 * c * d0 * d0
            c2 = b + 3 * c * d0
            qa = c1 - 0.5 * c * w * w
            qb = c2 + 1.5 * c * w
            kappa = k + j / S
            if abs(qa) + abs(qb) < 1e-12:
                continue
            phi = qa / (2 * qb) if qb != 0.0 else np.inf
            if np.isfinite(phi) and abs(phi) <= PHI_MAX:
                scale = np.sqrt(abs(qb))
                s_max = abs(qb) * max(phi * phi, (w + phi) * (w + phi))
                knots.append(dict(
                    kind="quad", kappa=kappa, w=w,
                    scale=float(scale), act_bias=float(scale * (phi - kappa)),
                    sign=1.0 if qb > 0 else -1.0, s_max=float(s_max),
                ))
                const -= qb * phi * phi
            else:
                coef = qa + qb * w
                knots.append(dict(kind="lin", kappa=kappa, w=w, coef=float(coef)))
                const -= coef * kappa
    return float(const), knots


def _assign_v1(knots):
    nq = sum(1 for k in knots if k["kind"] == "quad")
    order = sorted((i for i, k in enumerate(knots) if k["kind"] == "quad"),
                   key=lambda i: knots[i]["s_max"])
    budget = (2.0e-3 * 2048.0) ** 2
    acc2 = 0.0
    pe_r = set()
    for i in order:
        s2 = knots[i]["s_max"] ** 2
        if acc2 + s2 > budget:
            break
        acc2 += s2
        pe_r.add(i)
    if len(pe_r) == nq and nq > 0:
        pe_r.discard(order[-1])
    act_load = 3.404 * nq
    pe_load = 1.22 * len(pe_r)
    assign = {}
    for i, k in enumerate(knots):
        if k["kind"] == "quad":
            assign[i] = "pe_r" if i in pe_r else "stt"
    for i, k in enumerate(knots):
        if k["kind"] == "lin":
            if pe_load + 8.7 <= act_load:
                assign[i] = "pe_f"
                pe_load += 8.7
            else:
                assign[i] = "stt"
    if not any(v == "stt" for v in assign.values()):
        assign[0] = "stt"
    return assign


def _build_bass(const, knots, reps=1, act_f32r=True, assign=None):
    import concourse.bass as bass
    import concourse.tile as tile
    from concourse import mybir
    from collections import deque

    AL = mybir.AluOpType
    AF = mybir.ActivationFunctionType
    F32 = mybir.dt.float32
    F32R = mybir.dt.float32r
    nc = bass.Bass()
    x = nc.declare_dram_parameter("x", [P, FTOT], F32, isOutput=False)
    eye = nc.declare_dram_parameter("eye", [P, P], F32, isOutput=False)
    qb = nc.declare_dram_parameter("qb", [P, max(1, sum(1 for k in knots if k["kind"] == "quad"))], F32, isOutput=False)
    y = nc.declare_dram_parameter("y", [P, FTOT], F32, isOutput=True)
    NCH = TF // 512

    if assign is None:
        assign = _assign_v1(knots)
    pe_knots = [i for i in range(len(knots)) if assign[i] in ("pe_r", "pe_f")]
    pe_total = len(pe_knots)
    lin_pe = [i for i in pe_knots if assign[i] == "pe_f"]
    qcol = {}
    for i, kn in enumerate(knots):
        if kn["kind"] == "quad":
            qcol[i] = len(qcol)

    with tile.TileContext(nc) as tc:
        with (
            tc.tile_pool(name="cw", bufs=1) as cw,
            tc.tile_pool(name="xp", bufs=2) as xp,
            tc.tile_pool(name="cp2", bufs=3) as cpool,
            tc.tile_pool(name="spr", bufs=2) as spool_r,
            tc.tile_pool(name="spf", bufs=3) as spool_f,
            tc.tile_pool(name="yp", bufs=2) as ypool,
            tc.tile_pool(name="ap", bufs=2) as apool,
            tc.tile_pool(name="ps", bufs=1, space="PSUM") as psp,
        ):
            eyep = cw.tile([P, P], F32, tag="eyep")
            nc.sync.dma_start(eyep[:], eye[:])
            qbt = cw.tile([P, max(1, len(qcol))], F32, tag="qbt")
            nc.sync.dma_start(qbt[:], qb[:])
            eyer_p = cw.tile([P, P], F32R, tag="eyer_p")
            nc.vector.tensor_scalar(out=eyer_p[:], in0=eyep[:], scalar1=1.0, scalar2=None, op0=AL.mult)
            eyer_n = cw.tile([P, P], F32R, tag="eyer_n")
            nc.vector.tensor_scalar(out=eyer_n[:], in0=eyep[:], scalar1=-1.0, scalar2=None, op0=AL.mult)
            lin_w = {}
            for i in lin_pe:
                wt = cw.tile([P, P], F32, tag=f"w{i}")
                nc.vector.tensor_scalar(out=wt[:], in0=eyep[:],
                                        scalar1=float(knots[i]["coef"]), scalar2=None, op0=AL.mult)
                lin_w[i] = wt

            for it in [i % NT for i in range(NT * reps)]:
                xt = xp.tile([P, TF], F32, tag="xt")
                nc.sync.dma_start(xt[:], x[:, bass.ts(it, TF)])
                nc.vector.tensor_scalar(out=xt[:], in0=xt[:], scalar1=63.0, scalar2=None, op0=AL.mult)

                acc = apool.tile([P, TF], F32, tag="acc")
                if pe_total:
                    pacc = psp.tile([P, TF], F32, tag="pacc")
                else:
                    pacc = None
                pe_seen = 0
                pending = deque()
                first_dve_acc = True

                def flush_one():
                    nonlocal first_dve_acc
                    st, coef = pending.popleft()
                    if first_dve_acc:
                        nc.vector.tensor_scalar(
                            out=acc[:], in0=st[:], scalar1=coef, scalar2=float(const),
                            op0=AL.mult, op1=AL.add)
                        first_dve_acc = False
                    else:
                        nc.vector.scalar_tensor_tensor(
                            out=acc[:], in0=st[:], scalar=coef, in1=acc[:],
                            op0=AL.mult, op1=AL.add)

                for i, kn in enumerate(knots):
                    kap, w = kn["kappa"], kn["w"]
                    mode = assign[i]
                    ct = cpool.tile([P, TF], F32, tag="ct")
                    nc.vector.tensor_scalar(
                        out=ct[:], in0=xt[:], scalar1=float(kap), scalar2=float(kap + w),
                        op0=AL.max, op1=AL.min)
                    if kn["kind"] == "quad":
                        if mode == "pe_r":
                            st = spool_r.tile([P, TF], F32R, tag="str")
                            bias_ap = qbt[:, qcol[i] : qcol[i] + 1]
                            if act_f32r:
                                nc.scalar.activation(st[:], ct[:], AF.Square,
                                                     bias=bias_ap,
                                                     scale=float(kn["scale"]))
                            else:
                                sf = spool_f.tile([P, TF], F32, tag="stf")
                                nc.scalar.activation(sf[:], ct[:], AF.Square,
                                                     bias=bias_ap,
                                                     scale=float(kn["scale"]))
                                nc.vector.tensor_scalar(out=st[:], in0=sf[:],
                                                        scalar1=1.0, scalar2=None, op0=AL.mult)
                            pe_seen += 1
                            eyet = eyer_p if kn["sign"] > 0 else eyer_n
                            for jch in range(NCH):
                                nc.tensor.matmul(
                                    pacc[:, bass.ts(jch, 512)], eyet[:],
                                    st[:, bass.ts(jch, 512)],
                                    start=(pe_seen == 1), stop=(pe_seen == pe_total),
                                )
                        else:
                            st = spool_f.tile([P, TF], F32, tag="st")
                            nc.scalar.activation(st[:], ct[:], AF.Square,
                                                 bias=qbt[:, qcol[i] : qcol[i] + 1],
                                                 scale=float(kn["scale"]))
                            pending.append((st, float(kn["sign"])))
                    else:
                        if mode == "pe_f":
                            pe_seen += 1
                            for jch in range(NCH):
                                nc.tensor.matmul(
                                    pacc[:, bass.ts(jch, 512)], lin_w[i][:],
                                    ct[:, bass.ts(jch, 512)],
                                    start=(pe_seen == 1), stop=(pe_seen == pe_total),
                                )
                        else:
                            pending.append((ct, float(kn["coef"])))
                    while len(pending) > 2:
                        flush_one()
                while pending:
                    flush_one()

                if pe_total:
                    yt = ypool.tile([P, TF], F32, tag="yt")
                    nc.scalar.activation(yt[:], pacc[:], AF.Copy, bias=0.0, scale=1.0)
                    nc.vector.tensor_tensor(out=acc[:], in0=acc[:], in1=yt[:], op=AL.add)
                nc.sync.dma_start(y[:, bass.ts(it, TF)], acc[:])
    return nc


# ===========================================================================
# entry point
# ===========================================================================
def kernel(input, value):
    global LAST_EXEC_NS
    import time

    _apply_walrus_compat_patches()
    from concourse.bass_utils import run_bass_kernel_spmd

    input = np.ascontiguousarray(np.asarray(input, dtype=np.float32))
    value = np.ascontiguousarray(np.asarray(value, dtype=np.float32))
    key = value.tobytes()
    shards = input.reshape(N_CORES, P, FTOT)
    eyev = np.eye(P, dtype=np.float32)

    def _in_maps(nc, tables):
        names = set()
        from concourse import mybir as _mb
        for alloc in nc.m.functions[0].allocations:
            if isinstance(alloc, _mb.MemoryLocationSet) and alloc.kind == "ExternalInput":
                names.add(alloc.memorylocations[0].name)
        maps = []
        for c in range(N_CORES):
            m = {"x": shards[c]}
            for tname, tval in tables.items():
                if tname in names:
                    m[tname] = tval
            maps.append(m)
        return maps

    def _run(nc, tables):
        global LAST_EXEC_NS
        t0 = time.time()
        res = run_bass_kernel_spmd(nc, _in_maps(nc, tables), core_ids=list(range(N_CORES)))
        LAST_EXEC_NS = (time.time() - t0) * 1e9
        out = np.stack([res.results[c]["y"] for c in range(N_CORES)], axis=0)
        return out.reshape(B, CH, H, W).astype(np.float32, copy=False)

    entry = _CACHE.get(key)
    if entry is not None:
        return _run(*entry)

    candidates = []

    # v2 fp16 windowed-knot kernel
    def mk_v2():
        const, knots, ce, ae, _stats = plan_best(value)
        _eg, eb = verify_plan(value, const, knots, ae)
        if eb > ERR_GATE or not knots:
            raise ValueError(f"v2 plan error bound too large: {eb}")
        wstack, qb = make_tables(knots, ae)
        nc = build_v2(const, knots, ce, ae)
        return nc, {"wstack": wstack, "qb": qb}

    candidates.append(mk_v2)

    # v1 exact-telescoped fallback
    def mk_v1():
        const, knots = _plan2(value)
        qb_vals = np.asarray([k["act_bias"] for k in knots if k["kind"] == "quad"],
                             dtype=np.float32)
        if qb_vals.size == 0:
            qb_vals = np.zeros(1, np.float32)
        qb = np.broadcast_to(qb_vals, (P, qb_vals.size)).copy()
        nc = _build_bass(const, knots, act_f32r=True)
        return nc, {"eye": eyev, "qb": qb}

    candidates.append(mk_v1)

    last_exc = None
    for mk in candidates:
        try:
            nc, tables = mk()
            out = _run(nc, tables)
            _CACHE.clear()
            _CACHE[key] = (nc, tables)
            return out
        except Exception as e:
            last_exc = e
    raise last_exc


# revision 5
# speedup vs baseline: 3.0969x; 1.5479x over previous
"""Akima spline evaluation (nn_Akima_66623532696299) on 8 Trainium2 cores.

v2: fp16 windowed-knot decomposition, engine-balanced:

    y(x) = const + sum_j f_j(x16),   x16 = fp16(x) (gpsimd cast-DMA, RNE)
      lin knot:  f = g16 * clamp(x16, a, b)          DVE clamp (4x mode) +
                                                     PE fp16 diag-matmul acc
      quad knot: f = sgn * fp16((sig*c + bias)^2)    + ACT Square
    PSUM accumulates all knots in fp32; ACT drains PSUM + adds const; output
    written back as fp32 by a gpsimd casting DMA.

The host planner covers the 63 spline segments with pieces (1, 2, 0.5 or 0.25
segments wide, fp16 breakpoints) chosen by DP to minimize a per-engine cost,
with a per-piece error budget = TAU minus the local fp16 input-rounding
slack. The complete device pipeline (fp16 rounding included) is emulated
exactly on the full fp16 grid on the host; if the bound exceeds the safety
threshold the kernel falls back to the slower exact-telescoped v1 kernels.
"""

import numpy as np
import ml_dtypes

BF16 = ml_dtypes.bfloat16

N_CORES = 8
P = 128
B, CH, H, W = 64, 3, 512, 512
PER_CORE = (B // N_CORES) * CH * H * W        # 6291456
FTOT = PER_CORE // P                          # 49152
TF = 2048
NT = FTOT // TF                               # 24
NSEG = 63
TAU = 0.015
S_CAP = 1.0
ERR_GATE = 0.019

# per-op per-tile cost model ns (TF=2048) for engine balancing
C_CLAMP_DVE = 654.0
C_ACC_PE = 680.0
C_ACC_DVE = 2254.0
C_SQ_ACT = 2077.0
C_MERGE_DVE = 1187.0
C_CONV_DVE = 1187.0

_CACHE = {}
_QBIAS_CACHE = {0: None}
LAST_EXEC_NS = None


def _apply_walrus_compat_patches():
    """This container's walrus rejects >1 sync-wait command per instruction;
    Tile's wait assignment can emit several. Split excess waits onto bare
    same-engine NoOps committed immediately before the instruction."""
    import concourse.tile as tile
    from concourse import mybir
    from concourse.vector_clock import ScopedClock

    if getattr(tile.TileContext, "_akima_patched", False):
        return
    MAX_WAITS = 1
    _orig_commit = tile.TileContext._commit_instruction

    def _split_waits(self, inst, lazy_reg_writes=True):
        si = inst.sync_info
        if si is not None and si.on_wait and len(si.on_wait) > MAX_WAITS:
            waits = list(si.on_wait)
            updates = list(si.on_update or [])
            inst.sync_info = mybir.SyncInfo(on_wait=waits[:MAX_WAITS], on_update=updates)
            for i in range(MAX_WAITS, len(waits), MAX_WAITS):
                nop = mybir.InstNoOp(name=f"I-{self.nc.next_id()}", engine=inst.engine)
                nop.sync_info = mybir.SyncInfo(on_wait=waits[i : i + MAX_WAITS], on_update=[])
                _orig_commit(self, nop, lazy_reg_writes)
        return _orig_commit(self, inst, lazy_reg_writes)

    def _drain_and_barrier(self, tick_clock, wait_clock):
        nc = self.nc
        collector = nc.sync.nop(nofuse=True).ins
        wait_clock.add_sem_waits(collector, ScopedClock({None: tick_clock.global_clock}))
        si = collector.sync_info
        waits = list(si.on_wait or []) if si is not None else []
        updates = list(si.on_update or []) if si is not None else []
        if len(waits) > MAX_WAITS:
            collector.sync_info = mybir.SyncInfo(on_wait=waits[:MAX_WAITS], on_update=updates)
            rest = waits[MAX_WAITS:]
            while rest:
                extra = nc.sync.nop(nofuse=True).ins
                extra.sync_info = mybir.SyncInfo(on_wait=rest[:MAX_WAITS], on_update=[])
                rest = rest[MAX_WAITS:]
        nc.sync.drain()
        nc.all_engine_barrier()
        assert self.sems is not None
        popped = nc._tile_sem_poison_stack.pop()
        assert popped is self._sem_poison
        nc.clear_and_free_semaphores(list(self.sems.allocated().values()))
        nc.all_engine_barrier()

    tile.TileContext._commit_instruction = _split_waits
    tile.TileContext._drain_and_barrier = _drain_and_barrier
    tile.TileContext._akima_patched = True


# ===========================================================================
# v2: host planner
# ===========================================================================
def akima_locals(value):
    v = np.asarray(value, dtype=np.float64)
    n = v.shape[0]
    h = 1.0 / (n - 1)
    m = np.diff(v) / h
    m_ext = np.concatenate(
        [[3 * m[0] - 2 * m[1], 2 * m[0] - m[1]], m,
         [2 * m[-1] - m[-2], 3 * m[-1] - 2 * m[-2]]])
    dm = np.abs(np.diff(m_ext))
    w1 = dm[2:]
    w2 = dm[:-2]
    den = w1 + w2
    safe = np.where(den > 0, den, 1.0)
    ml = m_ext[1:n + 1]
    mr = m_ext[2:n + 2]
    t = np.where(den > 0, (w1 * ml + w2 * mr) / safe, 0.5 * (ml + mr))
    A = t[:-1] * h
    Bc = (3 * m - 2 * t[:-1] - t[1:]) * h
    Cc = (t[:-1] + t[1:] - 2 * m) * h
    return float(v[0]), A, Bc, Cc


def _mk_y(value):
    v0, A, Bc, Cc = akima_locals(value)
    vnodes = np.concatenate([[v0], v0 + np.cumsum(A + Bc + Cc)])

    def y(u):
        u = np.asarray(u, dtype=np.float64)
        uc = np.clip(u, 0.0, 63.0)
        idx = np.clip(np.floor(uc).astype(np.int64), 0, 62)
        d = uc - idx
        return vnodes[idx] + d * (A[idx] + d * (Bc[idx] + d * Cc[idx]))
    return y


def _f16_grid(a, b):
    ia = np.float16(a).view(np.uint16)
    ib = np.float16(b).view(np.uint16)
    codes = np.arange(int(ia), int(ib) + 1, dtype=np.uint16)
    return codes.view(np.float16).astype(np.float64)


def _piece_lin(y, a, b, L=None):
    """Linear knot over [a,b] starting from achieved level L (default y(63a)).
    Returns (g16, err, sat): sat = achieved contribution at saturation."""
    ya, yb = y(63 * a), y(63 * b)
    if L is None:
        L = ya
    gg = _f16_grid(a, b)
    cb = gg.astype(BF16).astype(np.float64)          # bf16 clamp output
    ab = float(np.float64(np.asarray(a).astype(BF16)))
    bb = float(np.float64(np.asarray(b).astype(BF16)))
    g = (yb - L) / (bb - ab) if bb > ab else 0.0
    g16 = float(np.float64(np.asarray(g).astype(BF16)))
    approx = L + g16 * (cb - ab)
    err = np.abs(approx - y(63 * gg)).max()
    sat = g16 * (bb - ab)
    return g16, float(err), float(sat)


def _piece_quad(y, a, b, L=None):
    Wd = b - a
    ya, yb = y(63 * a), y(63 * b)
    if L is None:
        L = ya
    dy = yb - L
    gg = _f16_grid(a, b)
    t = gg - a
    target = y(63 * gg) - L
    den = (t * t - Wd * t)
    num = target - (dy / Wd) * t
    denom = float((den * den).sum())
    beta0 = float((num * den).sum() / denom) if denom > 1e-30 else 0.0
    best = None
    for beta in np.linspace(beta0 - abs(beta0) - 1.0, beta0 + abs(beta0) + 1.0, 81):
        alpha = (dy - beta * Wd * Wd) / Wd
        if abs(beta) < 1e-9:
            continue
        sig = np.sqrt(abs(beta))
        hshift = alpha / (2 * beta)
        s_max = abs(beta) * max(hshift * hshift, (Wd + hshift) * (Wd + hshift))
        if s_max > S_CAP:
            continue
        sig32 = float(np.float32(sig))
        bias32 = float(np.float32(sig * (hshift - a)))
        sgn = 1.0 if beta > 0 else -1.0
        s = (np.float32(sig32) * gg.astype(np.float32)
             + np.float32(bias32)).astype(np.float32)
        s16 = (s * s).astype(BF16).astype(np.float64)
        s0 = float(np.asarray(np.float32((sig32 * np.float32(a) + bias32) ** 2)).astype(BF16))
        sb = float(np.asarray(np.float32((sig32 * np.float32(b) + bias32) ** 2)).astype(BF16))
        approx = sgn * s16 - sgn * s0
        err = np.abs(approx - target).max()
        sat = sgn * (sb - s0)
        if best is None or err < best[4]:
            best = (sig32, bias32, sgn, float(s_max), float(err), float(sat))
    return best


def plan(value, tau=TAU, quad_cost=3.2):
    y = _mk_y(value)

    def slack(a, b):
        gg = _f16_grid(a, b)
        if len(gg) < 3:
            return 0.0
        yv = y(63 * gg)
        dy = np.abs(np.diff(yv))
        halfd = np.concatenate([[dy[0]], np.maximum(dy[:-1], dy[1:]), [dy[-1]]]) / 2
        return float(halfd.max())

    def best_single(lo, hi):
        a, b = [float(np.float64(np.float16(lo / 63.0))), float(np.float64(np.float16(hi / 63.0)))]
        budget = tau - slack(a, b)
        if budget <= 0.001:
            return None
        g16, errl, _sat = _piece_lin(y, a, b)
        if errl <= budget:
            return 2.2, [dict(kind="lin", a=a, b=b, g=g16)]
        q = _piece_quad(y, a, b)
        if q is not None and q[4] <= budget:
            sig, bias, sgn, smax, _, _sat = q
            return quad_cost, [dict(kind="quad", a=a, b=b, sig=sig, bias=bias, sgn=sgn, smax=smax)]
        return None

    def cover(lo, hi, depth=0):
        s = best_single(lo, hi)
        if s is not None:
            return s
        if depth >= 3:
            a, b = [float(np.float64(np.float16(lo / 63.0))), float(np.float64(np.float16(hi / 63.0)))]
            q = _piece_quad(y, a, b)
            if q is None:
                g16, _e, _s = _piece_lin(y, a, b)
                return 2.2, [dict(kind="lin", a=a, b=b, g=g16)]
            sig, bias, sgn, smax, _, _sat = q
            return 3.2, [dict(kind="quad", a=a, b=b, sig=sig, bias=bias, sgn=sgn, smax=smax)]
        mid = (lo + hi) / 2
        c1, k1 = cover(lo, mid, depth + 1)
        c2, k2 = cover(mid, hi, depth + 1)
        return c1 + c2, k1 + k2

    INF = 1e18
    dp = [(INF, None)] * (NSEG + 1)
    dp[NSEG] = (0.0, None)
    choice = [None] * NSEG
    for k in range(NSEG - 1, -1, -1):
        cs, ks = cover(k, k + 1)
        best = (cs + dp[k + 1][0], ("single", ks))
        if k + 2 <= NSEG:
            p = best_single(k, k + 2)
            if p is not None:
                cp = p[0] + dp[k + 2][0]
                if cp < best[0]:
                    best = (cp, ("pair", p[1]))
        dp[k] = (best[0], best[1])
        choice[k] = best[1]
    knots = []
    k = 0
    while k < NSEG:
        kind, ks = choice[k]
        knots.extend(ks)
        k += 2 if kind == "pair" else 1

    # sequential refit with error feedback: each piece re-targets the
    # achieved level so bf16 saturation rounding does not random-walk
    L = y(0.0)
    for kn in knots:
        a, b = kn["a"], kn["b"]
        if kn["kind"] == "lin":
            g16, _e, sat = _piece_lin(y, a, b, L)
            kn["g"] = g16
            L += sat
        else:
            q = _piece_quad(y, a, b, L)
            if q is not None:
                sig, bias, sgn, smax, _e, sat = q
                kn.update(sig=sig, bias=bias, sgn=sgn, smax=smax)
                L += sat
            else:
                # keep original fit; track its sat
                sig32, bias32 = np.float32(kn["sig"]), np.float32(kn["bias"])
                s0 = float(np.asarray(np.float32((sig32 * np.float32(a) + bias32) ** 2)).astype(BF16))
                sb = float(np.asarray(np.float32((sig32 * np.float32(b) + bias32) ** 2)).astype(BF16))
                L += kn["sgn"] * (sb - s0)

    const = y(0.0)
    for kn in knots:
        if kn["kind"] == "lin":
            ab = float(np.asarray(kn["a"]).astype(BF16))
            const -= kn["g"] * ab
        else:
            const -= kn["sgn"] * float(np.asarray(
                np.float32((np.float32(kn["sig"]) * np.float32(kn["a"])
                            + np.float32(kn["bias"])) ** 2)).astype(BF16))
    return float(const), knots


def emulate(const, knots, acceng=None):
    gg = _f16_grid(0.0, 1.0)
    n = len(knots)
    if acceng is None:
        acceng = ["pe"] * n
    psum = np.zeros(gg.shape, dtype=np.float64)
    accD = None
    for j, kn in enumerate(knots):
        c = np.clip(gg, kn["a"], kn["b"])
        if kn["kind"] == "lin":
            src = c.astype(BF16).astype(np.float64)
            wv_pe = float(np.float64(np.asarray(kn["g"]).astype(BF16)))
            wv_dve = float(np.float32(kn["g"]))
        else:
            s = (np.float32(kn["sig"]) * c.astype(np.float32)
                 + np.float32(kn["bias"])).astype(np.float32)
            src = (s * s).astype(BF16).astype(np.float64)
            wv_pe = kn["sgn"]
            wv_dve = kn["sgn"]
        if acceng[j] == "pe":
            psum += wv_pe * src
        else:
            if accD is None:
                accD = (src * wv_dve).astype(np.float16).astype(np.float64)
            else:
                accD = (src * wv_dve + accD).astype(np.float16).astype(np.float64)
    yt = (psum + float(np.float32(const))).astype(np.float16).astype(np.float64)
    if accD is not None:
        yt = (yt + accD).astype(np.float16).astype(np.float64)
    return gg, yt


def verify_plan(value, const, knots, acceng=None):
    y = _mk_y(value)
    gg, yh = emulate(const, knots, acceng)
    ytrue = y(63 * gg)
    err_grid = np.abs(yh - ytrue)
    gaps = np.diff(gg)
    half = np.concatenate([[0], gaps]) / 2 + np.concatenate([gaps, [0]]) / 2
    dy = np.abs(np.gradient(ytrue, gg, edge_order=1))
    slack = dy * half
    return float(err_grid.max()), float((err_grid + slack).max())


def assign_engines(knots, conv_on_dve=False):
    n = len(knots)
    nsq = sum(1 for k in knots if k["kind"] == "quad")
    act = (nsq + 1) * C_SQ_ACT
    nlin = n - nsq
    best = None
    for aD in range(0, min(13, nlin + 1)):
        aP = n - aD
        dve = n * C_CLAMP_DVE + aD * C_ACC_DVE + (C_MERGE_DVE if aD else 0.0) \
            + (C_CONV_DVE if conv_on_dve else 0.0)
        pe = aP * C_ACC_PE
        t = max(dve, pe, act)
        if best is None or t < best[0]:
            best = (t, aD)
    t, aD = best
    clampeng = ["dve"] * n
    acceng = ["pe"] * n
    lin_idx = [j for j, k in enumerate(knots) if k["kind"] == "lin"]
    for j in lin_idx[:aD]:
        acceng[j] = "dve"
    return clampeng, acceng, dict(tile_ns=t, dve_accs=aD,
                                  act_ns=act, pe_ns=(n - aD) * C_ACC_PE)


def plan_best(value, tau=TAU):
    best = None
    for qc in (2.6, 3.2, 4.2, 5.6, 7.5):
        const, knots = plan(value, tau=tau, quad_cost=qc)
        ce, ae, stats = assign_engines(knots)
        if best is None or stats["tile_ns"] < best[3]["tile_ns"]:
            best = (const, knots, (ce, ae), stats, qc)
    const, knots, (ce, ae), stats, qc = best
    return const, knots, ce, ae, dict(stats, quad_cost=qc)


def make_tables(knots, acceng):
    pe_knots = [j for j in range(len(knots)) if acceng[j] == "pe"]
    n_pe = len(pe_knots)
    wstack = np.zeros((P, max(1, n_pe) * P), BF16)
    for i, j in enumerate(pe_knots):
        kn = knots[j]
        wv = kn["sgn"] if kn["kind"] == "quad" else kn["g"]
        blk = wstack[:, i * P:(i + 1) * P]
        np.fill_diagonal(blk, np.asarray(wv).astype(BF16))
    quads = [j for j in range(len(knots)) if knots[j]["kind"] == "quad"]
    qb = np.zeros((P, max(1, len(quads))), np.float32)
    for i, j in enumerate(quads):
        qb[:, i] = np.float32(knots[j]["bias"])
    return wstack, qb


# ===========================================================================
# v2: bass builder
# ===========================================================================
def build_v2(const, knots, clampeng, acceng, reps=1):
    import concourse.bass as bass
    import concourse.tile as tile
    from concourse import mybir

    AL = mybir.AluOpType
    AF = mybir.ActivationFunctionType
    F32 = mybir.dt.float32
    F16 = mybir.dt.float16
    B16 = mybir.dt.bfloat16

    n = len(knots)
    pe_knots = [j for j in range(n) if acceng[j] == "pe"]
    n_pe = len(pe_knots)
    quads = [j for j in range(n) if knots[j]["kind"] == "quad"]
    qcol = {j: i for i, j in enumerate(quads)}
    pecol = {j: i for i, j in enumerate(pe_knots)}

    nc = bass.Bass()
    x = nc.declare_dram_parameter("x", [P, FTOT], F32, isOutput=False)
    wstack = nc.declare_dram_parameter("wstack", [P, max(1, n_pe) * P], B16, isOutput=False)
    qb = nc.declare_dram_parameter("qb", [P, max(1, len(quads))], F32, isOutput=False)
    y = nc.declare_dram_parameter("y", [P, FTOT], F32, isOutput=True)

    NCH = TF // 512

    with tile.TileContext(nc) as tc:
        with (
            tc.tile_pool(name="cw", bufs=1) as cw,
            tc.tile_pool(name="xp", bufs=3) as xp,
            tc.tile_pool(name="cp", bufs=6) as cpool,
            tc.tile_pool(name="ca", bufs=3) as capool,
            tc.tile_pool(name="sp", bufs=4) as spool,
            tc.tile_pool(name="ap", bufs=2) as apool,
            tc.tile_pool(name="yp", bufs=3) as ypool,
            tc.tile_pool(name="ps", bufs=2, space="PSUM") as psp,
        ):
            wst = cw.tile([P, max(1, n_pe) * P], B16, tag="wst")
            nc.sync.dma_start(wst[:], wstack[:])
            qbt = cw.tile([P, max(1, len(quads))], F32, tag="qbt")
            nc.sync.dma_start(qbt[:], qb[:])

            for it in [i % NT for i in range(NT * reps)]:
                x16 = xp.tile([P, TF], F16, tag="x16")
                nc.gpsimd.dma_start(x16[:], x[:, bass.ts(it, TF)])
                pacc = psp.tile([P, TF], F32, tag="pacc")
                accD = None
                pe_seen = 0
                for j, kn in enumerate(knots):
                    is_dve_acc = acceng[j] == "dve"
                    pool = capool if is_dve_acc else cpool
                    cdt = F16 if kn["kind"] == "quad" else B16
                    c = pool.tile([P, TF], cdt, tag="ca" if is_dve_acc else "c")
                    eng = nc.vector if clampeng[j] == "dve" else nc.gpsimd
                    eng.tensor_scalar(out=c[:], in0=x16[:],
                                      scalar1=float(kn["a"]), scalar2=float(kn["b"]),
                                      op0=AL.max, op1=AL.min)
                    if kn["kind"] == "quad":
                        s = spool.tile([P, TF], B16, tag="s")
                        nc.scalar.activation(s[:], c[:], AF.Square,
                                             bias=qbt[:, qcol[j]:qcol[j] + 1],
                                             scale=float(kn["sig"]))
                        src = s
                        wv = kn["sgn"]
                    else:
                        src = c
                        wv = kn["g"]
                    if acceng[j] == "pe":
                        pj = pecol[j]
                        for jch in range(NCH):
                            nc.tensor.matmul(
                                pacc[:, bass.ts(jch, 512)],
                                wst[:, bass.ts(pj, P)],
                                src[:, bass.ts(jch, 512)],
                                start=(pe_seen == 0), stop=(pe_seen == n_pe - 1))
                        pe_seen += 1
                    else:
                        if accD is None:
                            accD = apool.tile([P, TF], B16, tag="accD")
                            nc.vector.tensor_scalar(out=accD[:], in0=src[:],
                                                    scalar1=float(wv), scalar2=None,
                                                    op0=AL.mult)
                        else:
                            nc.vector.scalar_tensor_tensor(
                                out=accD[:], in0=src[:], scalar=float(wv),
                                in1=accD[:], op0=AL.mult, op1=AL.add)
                yt = ypool.tile([P, TF], F16, tag="yt")
                nc.scalar.activation(yt[:], pacc[:], AF.Copy, bias=float(const),
                                     scale=1.0)
                if accD is not None:
                    nc.vector.tensor_tensor(out=yt[:], in0=yt[:], in1=accD[:],
                                            op=AL.add)
                    accD = None
                nc.gpsimd.dma_start(y[:, bass.ts(it, TF)], yt[:])
    return nc


# ===========================================================================
# v1 fallback kernels (exact-telescoped quadratic / cubic)
# ===========================================================================
def _plan2(value):
    MAXL = 3
    TARGET_ABS = 5.0e-3
    PHI_MAX = 4.0
    v0, A, Bc, Cc = akima_locals(value)
    CHEB = 1.0 / (12.0 * np.sqrt(3.0))
    knots = []
    const = v0
    for k in range(len(A)):
        a, b, c = float(A[k]), float(Bc[k]), float(Cc[k])
        if abs(a) + abs(b) + abs(c) < 1e-12:
            continue
        L = 0
        while L < MAXL and CHEB * abs(c) / (8.0 ** L) > TARGET_ABS:
            L += 1
        S = 1 << L
        w = 1.0 / S
        for j in range(S):
            d0 = j / S
            c1 = a + 2 * b * d0 + 3 * c * d0 * d0
            c2 = b + 3 * c * d0
            qa = c1 - 0.5 * c * w * w
            qb = c2 + 1.5 * c * w
            kappa = k + j / S
            if abs(qa) + abs(qb) < 1e-12:
                continue
            phi = qa / (2 * qb) if qb != 0.0 else np.inf
            if np.isfinite(phi) and abs(phi) <= PHI_MAX:
                scale = np.sqrt(abs(qb))
                s_max = abs(qb) * max(phi * phi, (w + phi) * (w + phi))
                knots.append(dict(
                    kind="quad", kappa=kappa, w=w,
                    scale=float(scale), act_bias=float(scale * (phi - kappa)),
                    sign=1.0 if qb > 0 else -1.0, s_max=float(s_max),
                ))
                const -= qb * phi * phi
            else:
                coef = qa + qb * w
                knots.append(dict(kind="lin", kappa=kappa, w=w, coef=float(coef)))
                const -= coef * kappa
    return float(const), knots


def _assign_v1(knots):
    nq = sum(1 for k in knots if k["kind"] == "quad")
    order = sorted((i for i, k in enumerate(knots) if k["kind"] == "quad"),
                   key=lambda i: knots[i]["s_max"])
    budget = (2.0e-3 * 2048.0) ** 2
    acc2 = 0.0
    pe_r = set()
    for i in order:
        s2 = knots[i]["s_max"] ** 2
        if acc2 + s2 > budget:
            break
        acc2 += s2
        pe_r.add(i)
    if len(pe_r) == nq and nq > 0:
        pe_r.discard(order[-1])
    act_load = 3.404 * nq
    pe_load = 1.22 * len(pe_r)
    assign = {}
    for i, k in enumerate(knots):
        if k["kind"] == "quad":
            assign[i] = "pe_r" if i in pe_r else "stt"
    for i, k in enumerate(knots):
        if k["kind"] == "lin":
            if pe_load + 8.7 <= act_load:
                assign[i] = "pe_f"
                pe_load += 8.7
            else:
                assign[i] = "stt"
    if not any(v == "stt" for v in assign.values()):
        assign[0] = "stt"
    return assign


def _build_bass(const, knots, reps=1, act_f32r=True, assign=None):
    import concourse.bass as bass
    import concourse.tile as tile
    from concourse import mybir
    from collections import deque

    AL = mybir.AluOpType
    AF = mybir.ActivationFunctionType
    F32 = mybir.dt.float32
    F32R = mybir.dt.float32r
    nc = bass.Bass()
    x = nc.declare_dram_parameter("x", [P, FTOT], F32, isOutput=False)
    eye = nc.declare_dram_parameter("eye", [P, P], F32, isOutput=False)
    qb = nc.declare_dram_parameter("qb", [P, max(1, sum(1 for k in knots if k["kind"] == "quad"))], F32, isOutput=False)
    y = nc.declare_dram_parameter("y", [P, FTOT], F32, isOutput=True)
    NCH = TF // 512

    if assign is None:
        assign = _assign_v1(knots)
    pe_knots = [i for i in range(len(knots)) if assign[i] in ("pe_r", "pe_f")]
    pe_total = len(pe_knots)
    lin_pe = [i for i in pe_knots if assign[i] == "pe_f"]
    qcol = {}
    for i, kn in enumerate(knots):
        if kn["kind"] == "quad":
            qcol[i] = len(qcol)

    with tile.TileContext(nc) as tc:
        with (
            tc.tile_pool(name="cw", bufs=1) as cw,
            tc.tile_pool(name="xp", bufs=2) as xp,
            tc.tile_pool(name="cp2", bufs=3) as cpool,
            tc.tile_pool(name="spr", bufs=2) as spool_r,
            tc.tile_pool(name="spf", bufs=3) as spool_f,
            tc.tile_pool(name="yp", bufs=2) as ypool,
            tc.tile_pool(name="ap", bufs=2) as apool,
            tc.tile_pool(name="ps", bufs=1, space="PSUM") as psp,
        ):
            eyep = cw.tile([P, P], F32, tag="eyep")
            nc.sync.dma_start(eyep[:], eye[:])
            qbt = cw.tile([P, max(1, len(qcol))], F32, tag="qbt")
            nc.sync.dma_start(qbt[:], qb[:])
            eyer_p = cw.tile([P, P], F32R, tag="eyer_p")
            nc.vector.tensor_scalar(out=eyer_p[:], in0=eyep[:], scalar1=1.0, scalar2=None, op0=AL.mult)
            eyer_n = cw.tile([P, P], F32R, tag="eyer_n")
            nc.vector.tensor_scalar(out=eyer_n[:], in0=eyep[:], scalar1=-1.0, scalar2=None, op0=AL.mult)
            lin_w = {}
            for i in lin_pe:
                wt = cw.tile([P, P], F32, tag=f"w{i}")
                nc.vector.tensor_scalar(out=wt[:], in0=eyep[:],
                                        scalar1=float(knots[i]["coef"]), scalar2=None, op0=AL.mult)
                lin_w[i] = wt

            for it in [i % NT for i in range(NT * reps)]:
                xt = xp.tile([P, TF], F32, tag="xt")
                nc.sync.dma_start(xt[:], x[:, bass.ts(it, TF)])
                nc.vector.tensor_scalar(out=xt[:], in0=xt[:], scalar1=63.0, scalar2=None, op0=AL.mult)

                acc = apool.tile([P, TF], F32, tag="acc")
                if pe_total:
                    pacc = psp.tile([P, TF], F32, tag="pacc")
                else:
                    pacc = None
                pe_seen = 0
                pending = deque()
                first_dve_acc = True

                def flush_one():
                    nonlocal first_dve_acc
                    st, coef = pending.popleft()
                    if first_dve_acc:
                        nc.vector.tensor_scalar(
                            out=acc[:], in0=st[:], scalar1=coef, scalar2=float(const),
                            op0=AL.mult, op1=AL.add)
                        first_dve_acc = False
                    else:
                        nc.vector.scalar_tensor_tensor(
                            out=acc[:], in0=st[:], scalar=coef, in1=acc[:],
                            op0=AL.mult, op1=AL.add)

                for i, kn in enumerate(knots):
                    kap, w = kn["kappa"], kn["w"]
                    mode = assign[i]
                    ct = cpool.tile([P, TF], F32, tag="ct")
                    nc.vector.tensor_scalar(
                        out=ct[:], in0=xt[:], scalar1=float(kap), scalar2=float(kap + w),
                        op0=AL.max, op1=AL.min)
                    if kn["kind"] == "quad":
                        if mode == "pe_r":
                            st = spool_r.tile([P, TF], F32R, tag="str")
                            bias_ap = qbt[:, qcol[i] : qcol[i] + 1]
                            if act_f32r:
                                nc.scalar.activation(st[:], ct[:], AF.Square,
                                                     bias=bias_ap,
                                                     scale=float(kn["scale"]))
                            else:
                                sf = spool_f.tile([P, TF], F32, tag="stf")
                                nc.scalar.activation(sf[:], ct[:], AF.Square,
                                                     bias=bias_ap,
                                                     scale=float(kn["scale"]))
                                nc.vector.tensor_scalar(out=st[:], in0=sf[:],
                                                        scalar1=1.0, scalar2=None, op0=AL.mult)
                            pe_seen += 1
                            eyet = eyer_p if kn["sign"] > 0 else eyer_n
                            for jch in range(NCH):
                                nc.tensor.matmul(
                                    pacc[:, bass.ts(jch, 512)], eyet[:],
                                    st[:, bass.ts(jch, 512)],
                                    start=(pe_seen == 1), stop=(pe_seen == pe_total),
                                )
                        else:
                            st = spool_f.tile([P, TF], F32, tag="st")
                            nc.scalar.activation(st[:], ct[:], AF.Square,
                                                 bias=qbt[:, qcol[i] : qcol[i] + 1],
                                                 scale=float(kn["scale"]))
                            pending.append((st, float(kn["sign"])))
                    else:
                        if mode == "pe_f":
                            pe_seen += 1
                            for jch in range(NCH):
                                nc.tensor.matmul(
                                    pacc[:, bass.ts(jch, 512)], lin_w[i][:],
                                    ct[:, bass.ts(jch, 512)],
                                    start=(pe_seen == 1), stop=(pe_seen == pe_total),
                                )
                        else:
                            pending.append((ct, float(kn["coef"])))
                    while len(pending) > 2:
                        flush_one()
                while pending:
                    flush_one()

                if pe_total:
                    yt = ypool.tile([P, TF], F32, tag="yt")
                    nc.scalar.activation(yt[:], pacc[:], AF.Copy, bias=0.0, scale=1.0)
                    nc.vector.tensor_tensor(out=acc[:], in0=acc[:], in1=yt[:], op=AL.add)
                nc.sync.dma_start(y[:, bass.ts(it, TF)], acc[:])
    return nc


# ===========================================================================
# entry point
# ===========================================================================
def kernel(input, value):
    global LAST_EXEC_NS
    import time

    _apply_walrus_compat_patches()
    from concourse.bass_utils import run_bass_kernel_spmd

    input = np.ascontiguousarray(np.asarray(input, dtype=np.float32))
    value = np.ascontiguousarray(np.asarray(value, dtype=np.float32))
    key = value.tobytes()
    shards = input.reshape(N_CORES, P, FTOT)
    eyev = np.eye(P, dtype=np.float32)

    def _in_maps(nc, tables):
        names = set()
        from concourse import mybir as _mb
        for alloc in nc.m.functions[0].allocations:
            if isinstance(alloc, _mb.MemoryLocationSet) and alloc.kind == "ExternalInput":
                names.add(alloc.memorylocations[0].name)
        maps = []
        for c in range(N_CORES):
            m = {"x": shards[c]}
            for tname, tval in tables.items():
                if tname in names:
                    m[tname] = tval
            maps.append(m)
        return maps

    def _run(nc, tables):
        global LAST_EXEC_NS
        t0 = time.time()
        res = run_bass_kernel_spmd(nc, _in_maps(nc, tables), core_ids=list(range(N_CORES)))
        LAST_EXEC_NS = (time.time() - t0) * 1e9
        out = np.stack([res.results[c]["y"] for c in range(N_CORES)], axis=0)
        return out.reshape(B, CH, H, W).astype(np.float32, copy=False)

    entry = _CACHE.get(key)
    if entry is not None:
        return _run(*entry)

    candidates = []

    # v2 fp16 windowed-knot kernel
    def mk_v2():
        const, knots, ce, ae, _stats = plan_best(value)
        _eg, eb = verify_plan(value, const, knots, ae)
        if eb > ERR_GATE or not knots:
            raise ValueError(f"v2 plan error bound too large: {eb}")
        wstack, qb = make_tables(knots, ae)
        nc = build_v2(const, knots, ce, ae)
        return nc, {"wstack": wstack, "qb": qb}

    candidates.append(mk_v2)

    # v1 exact-telescoped fallback
    def mk_v1():
        const, knots = _plan2(value)
        qb_vals = np.asarray([k["act_bias"] for k in knots if k["kind"] == "quad"],
                             dtype=np.float32)
        if qb_vals.size == 0:
            qb_vals = np.zeros(1, np.float32)
        qb = np.broadcast_to(qb_vals, (P, qb_vals.size)).copy()
        nc = _build_bass(const, knots, act_f32r=True)
        return nc, {"eye": eyev, "qb": qb}

    candidates.append(mk_v1)

    last_exc = None
    for mk in candidates:
        try:
            nc, tables = mk()
            out = _run(nc, tables)
            _CACHE.clear()
            _CACHE[key] = (nc, tables)
            return out
        except Exception as e:
            last_exc = e
    raise last_exc
